# revision 12
# baseline (speedup 1.0000x reference)
"""Trainium2 Bass kernel for nn_Net_18966575579675 (dense_mlp).

722 independent tiny MLPs: per net n, per batch row b:
  x = [prior[b,n], camera[b,:]]            (11)
  h1 = relu(x @ W1[n] + b1[n])             (10)
  h2 = relu(h1 @ W2[n] + b2[n])            (10)
  out[b,n,:] = h2 @ W3[n] + b3[n]          (6)

Strategy: data-parallel over batch (8 cores x 1024 rows). On each core the
PE runs in 32x32 tiled mode (16 independent sub-arrays). Each tile handles
a "triple" (3 nets) as a block-diagonal matmul:
  L1: K=23 (10 cam + 12 prior-of-rowgroup + ones), M=30, zero-padding in
      the weight block selects the tile's own 3 prior rows.
  L2: K=31 (30 h1 + ones), M=30.
  L3: K=31 (30 h2 + ones), M=18 (bf16).
Biases ride on a constant ones-row. Relu is fused into the PSUM->SBUF
eviction (DVE tensor_scalar_max / ACT activation Relu).
Output leaves the device as [triple*18+v, batch]; the host transposes
during the unshard step.
"""

import sys

sys.path.insert(0, "/opt/trn_rl_repo")

import numpy as np
import ml_dtypes

import concourse.bass as bass
import concourse.bacc as bacc
import concourse.mybir as mybir
from concourse import tile

F32 = mybir.dt.float32
F32R = mybir.dt.float32r
BF16 = mybir.dt.bfloat16

# Problem constants
B = 8192
N_NETS = 722
CAM = 10
H = 10
LOD = 6
N_CORES = 8

# Kernel geometry (full-size)
BC = B // N_CORES      # batch per core (1024)
NSG = 16               # supergroups of 16 triples
NT = NSG * 16          # triples total (256 -> 768 nets, padded)
NQ = NSG * 4           # quads
NPAD = NT * 3          # padded net count (768)


def _pad_nets(a, npad):
    """Pad (or slice) axis 0 (net axis) with zeros up to npad."""
    if npad <= a.shape[0]:
        return a[:npad]
    pad = [(0, npad - a.shape[0])] + [(0, 0)] * (a.ndim - 1)
    return np.pad(a, pad)


def build_host_tensors(prior_lod, camera, W1, b1, W2, b2, W3, b3,
                       bc=BC, nsg=NSG, n_cores=N_CORES):
    """Build per-core DRAM input tensors (numpy). Returns list of dicts."""
    nt = nsg * 16
    nq = nsg * 4
    npad = nt * 3

    W1 = _pad_nets(np.asarray(W1, np.float32), npad)
    b1 = _pad_nets(np.asarray(b1, np.float32), npad)
    W2 = _pad_nets(np.asarray(W2, np.float32), npad)
    b2 = _pad_nets(np.asarray(b2, np.float32), npad)
    W3 = _pad_nets(np.asarray(W3, np.float32), npad)
    b3 = _pad_nets(np.asarray(b3, np.float32), npad)
    prior = np.asarray(prior_lod, np.float32)
    camera = np.asarray(camera, np.float32)

    # ---- weights (shared by all cores) ----
    # W1H[i, k, g, j, m]: rowgroup i, K=23 rows, supergroup g, tile j, M=30
    W1H = np.zeros((4, 23, nsg, 4, 32), np.float32)
    W1H[:, 22, :, :, 30] = 1.0  # ones-row passthrough keeps h1 row 30 == 1
    # triple index T = 16g + 4i + j ; nets 3T+u ; M col = 10u + h
    g_ = np.arange(nsg)[:, None, None]          # g
    i_ = np.arange(4)[None, :, None]            # i
    j_ = np.arange(4)[None, None, :]            # j
    T_ = 16 * g_ + 4 * i_ + j_                  # [g,i,j]
    for u in range(3):
        n_idx = 3 * T_ + u                      # [g,i,j]
        # cam rows 0..9: W1H[i, c, g, j, 10u+h] = W1[n, 1+c, h]
        blk = W1[n_idx, 1:, :]                  # [g,i,j,10,10]
        for c in range(CAM):
            W1H[:, c, :, :, 10 * u:10 * u + 10] = np.transpose(
                blk[:, :, :, c, :], (1, 0, 2, 3))
        # prior row 10 + 3j + u: weight W1[n, 0, h]
        pr = W1[n_idx, 0, :]                    # [g,i,j,10]
        for jj in range(4):
            W1H[:, 10 + 3 * jj + u, :, jj, 10 * u:10 * u + 10] = np.transpose(
                pr[:, :, jj, :], (1, 0, 2))
        # ones row 22: bias
        W1H[:, 22, :, :, 10 * u:10 * u + 10] = np.transpose(
            b1[n_idx], (1, 0, 2, 3))
    W1H = np.ascontiguousarray(W1H.reshape(4, 23, nsg * 4 * 32)).astype(
        ml_dtypes.bfloat16)

    # W2H[t, k, q, m]: rowgroup t (triple-in-quad), K=31, quad q, M=30
    W2H = np.zeros((4, 31, nq, 32), np.float32)
    W2H[:, 30, :, 30] = 1.0
    q_ = np.arange(nq)[None, :]
    t_ = np.arange(4)[:, None]
    T2 = 4 * q_ + t_                            # [t, q]
    for u in range(3):
        n_idx = 3 * T2 + u                      # [t, q]
        w = W2[n_idx]                           # [t, q, 10, 10]
        for h in range(H):
            W2H[:, 10 * u + h, :, 10 * u:10 * u + 10] = w[:, :, h, :]
        W2H[:, 30, :, 10 * u:10 * u + 10] = b2[n_idx]
    W2H = np.ascontiguousarray(W2H.reshape(4, 31, nq * 32)).astype(
        ml_dtypes.bfloat16)

    # W3H[w, k, T, m]: replicated across rowgroups w, K=31, M=18, bf16
    W3H = np.zeros((31, nt, 32), np.float32)
    T3 = np.arange(nt)
    for u in range(3):
        n_idx = 3 * T3 + u
        w = W3[n_idx]                           # [T, 10, 6]
        for h in range(H):
            W3H[10 * u + h, :, 6 * u:6 * u + 6] = w[:, h, :]
        W3H[30, :, 6 * u:6 * u + 6] = b3[n_idx]
    W3H = np.broadcast_to(W3H[None], (4, 31, nt, 32))
    W3H = np.ascontiguousarray(W3H.reshape(4, 31, nt * 32)).astype(
        ml_dtypes.bfloat16)

    # ---- per-core xin ----
    in_maps = []
    for core in range(n_cores):
        b0 = core * bc
        pr = prior[b0:b0 + bc]                  # [bc, 722]
        cam = camera[b0:b0 + bc]                # [bc, 10]
        xin = np.zeros((4, 23, nsg, bc), np.float32)
        xin[:, 0:10] = cam.T[None, :, None, :]
        xin[:, 22] = 1.0
        # prior rows 10 + 3j + u -> net 3*(16g+4i+j)+u
        for i in range(4):
            for jj in range(4):
                for u in range(3):
                    nets = 3 * (16 * np.arange(nsg) + 4 * i + jj) + u  # [g]
                    valid = nets < N_NETS
                    rows = np.zeros((nsg, bc), np.float32)
                    rows[valid] = pr[:, nets[valid]].T
                    xin[i, 10 + 3 * jj + u] = rows
        xin = np.ascontiguousarray(xin.reshape(4, 23, nsg * bc)).astype(
            ml_dtypes.bfloat16)
        in_maps.append({"xin": xin, "w1": W1H, "w2": W2H, "w3": W3H})
    return in_maps


def build_program(bc=BC, nsg=NSG, reps=0):
    """Build the per-core Bass program (SPMD; identical on all cores).

    reps>0 wraps the whole body in a For_i repeat loop (timing builds only).
    """
    nt = nsg * 16
    nq = nsg * 4
    hc = bc // 2   # chunk size (L1 free dim)
    qc = bc // 4   # window size (L2/L3 free dim)

    nc = bacc.Bacc(None)
    xin_d = nc.dram_tensor("xin", [4, 23, nsg * bc], BF16, kind="ExternalInput")
    w1_d = nc.dram_tensor("w1", [4, 23, nsg * 4 * 32], BF16, kind="ExternalInput")
    w2_d = nc.dram_tensor("w2", [4, 31, nq * 32], BF16, kind="ExternalInput")
    w3_d = nc.dram_tensor("w3", [4, 31, nt * 32], BF16, kind="ExternalInput")
    ot_d = nc.dram_tensor("OT", [nt * 18, bc], F32, kind="ExternalOutput")
    # view for the output DMA: (g, pr, p, t, v) rows, (w, c) cols
    ot_v = ot_d[:].rearrange(
        "(g pr p t v) (w c) -> g pr t v w p c",
        g=nsg, pr=2, p=2, t=4, v=18, w=4, c=qc)

    import contextlib
    with tile.TileContext(nc) as tc:
        with tc.tile_pool(name="fix", bufs=1) as fix, \
             tc.tile_pool(name="psum", bufs=2, space="PSUM") as pp, \
             (tc.For_i(0, reps, 1) if reps else contextlib.nullcontext()):
            X = fix.tile([128, nsg * bc], BF16, tag="X")
            W1s = fix.tile([128, nsg * 4 * 32], BF16, tag="W1s")
            W2s = fix.tile([128, nq * 32], BF16, tag="W2s")
            W3s = fix.tile([128, nt * 32], BF16, tag="W3s")
            h1 = [fix.tile([128, 4 * bc], BF16, tag=f"h1{x}", name=f"h1{x}") for x in "ab"]
            h2 = [fix.tile([128, 2 * bc], BF16, tag=f"h2{x}", name=f"h2{x}") for x in "ab"]
            osb = [fix.tile([128, 2 * bc], F32, tag=f"osb{x}", name=f"osb{x}") for x in "ab"]

            # ---- input DMAs ----
            for i in range(4):
                nc.sync.dma_start(out=X[32 * i:32 * i + 23, :], in_=xin_d[i])
                nc.sync.dma_start(out=W1s[32 * i:32 * i + 23, :], in_=w1_d[i])
                nc.sync.dma_start(out=W2s[32 * i:32 * i + 31, :], in_=w2_d[i])
                nc.sync.dma_start(out=W3s[32 * i:32 * i + 31, :], in_=w3_d[i])

            # ---- main loop over supergroups ----
            for g in range(nsg):
                pg = g % 2
                h1sb, h2sb_, osb_ = h1[pg], h2, osb

                # L1: two chunks, 16 tiles each
                Tts = []
                for c in range(2):
                    Tt = pp.tile([128, 2048], F32, tag="ps", name="psT1")
                    Tts.append(Tt)
                    for i in range(4):
                        for j in range(4):
                            nc.tensor.matmul(
                                Tt[32 * j:32 * j + 32,
                                   512 * i:512 * i + hc],
                                W1s[32 * i:32 * i + 23,
                                    (g * 4 + j) * 32:(g * 4 + j) * 32 + 32
                                    ],
                                X[32 * i:32 * i + 23,
                                  g * bc + c * hc:g * bc + (c + 1) * hc
                                  ],
                                tile_position=(32 * i, 32 * j),
                            )
                # L1 evict (relu) -> h1sb ; engine alternates per chunk
                for c, Tt in enumerate(Tts):
                    src = Tt[:, :].rearrange("p (i x) -> p i x", i=4)[
                        :, :, 0:hc]
                    dst = h1sb[:, :].rearrange("p (i x) -> p i x", i=4)[
                        :, :, c * hc:(c + 1) * hc]
                    if c == 0:
                        nc.vector.tensor_scalar_max(dst, src, 0.0)
                    else:
                        nc.scalar.activation(
                            dst, src, mybir.ActivationFunctionType.Relu)

                # L2: 4 quads -> 2 psum tiles (pair: p = q%2)
                for pair in range(2):
                    T2t = pp.tile([128, 2048], F32, tag="ps", name="psT2")
                    for p in range(2):
                        i_blk = 2 * pair + p
                        q = 4 * g + i_blk
                        for t in range(4):
                            for w in range(4):
                                nc.tensor.matmul(
                                    T2t[32 * w:32 * w + 32,
                                        512 * t + qc * p:512 * t + qc * p + qc],
                                    W2s[32 * t:32 * t + 31,
                                        q * 32:q * 32 + 32],
                                    h1sb[32 * t:32 * t + 31,
                                         i_blk * bc + w * qc:
                                         i_blk * bc + (w + 1) * qc
                                         ],
                                    tile_position=(32 * t, 32 * w),
                                )
                    # evict (relu, ->bf16) -> h2sb[pair]
                    src = T2t[:, :].rearrange("p (t x) -> p t x", t=4)[
                        :, :, 0:hc]
                    dst = h2sb_[pair][:, :].rearrange(
                        "p (t x) -> p t x", t=4)[:, :, 0:hc]
                    if pair == 0:
                        nc.vector.tensor_scalar_max(dst, src, 0.0)
                    else:
                        nc.scalar.activation(
                            dst, src, mybir.ActivationFunctionType.Relu)

                # L3: per pair, 8 triples (t, p) x 4 windows, bf16
                for pair in range(2):
                    T3t = pp.tile([128, 2048], F32, tag="ps", name="psT3")
                    for p in range(2):
                        for t in range(4):
                            T_g = 4 * (4 * g + 2 * pair + p) + t
                            for w in range(4):
                                nc.tensor.matmul(
                                    T3t[32 * t:32 * t + 32,
                                        512 * w + qc * p:512 * w + qc * p + qc],
                                    W3s[32 * w:32 * w + 31,
                                        T_g * 32:T_g * 32 + 32],
                                    h2sb_[pair][32 * w:32 * w + 31,
                                                hc * t + qc * p:
                                                hc * t + qc * p + qc],
                                    tile_position=(32 * w, 32 * t),
                                )
                    # evict (copy) -> osb[pair]
                    src = T3t[:, :].rearrange("p (w x) -> p w x", w=4)[
                        :, :, 0:hc]
                    dst = osb_[pair][:, :].rearrange(
                        "p (w x) -> p w x", w=4)[:, :, 0:hc]
                    if pair == 0:
                        nc.vector.tensor_scalar_add(dst, src, 0.0)
                    else:
                        nc.scalar.activation(
                            dst, src, mybir.ActivationFunctionType.Copy)

                    # output DMA: per t, [18, w, p, c]
                    for t in range(4):
                        src_d = osb_[pair][32 * t:32 * t + 18, :].rearrange(
                            "v (w p c) -> v w p c", w=4, p=2)
                        nc.sync.dma_start(
                            out=ot_v[g, pair, t], in_=src_d)
    nc.finalize()
    return nc


def _unshard(results, bc=BC, nsg=NSG):
    nt = nsg * 16
    out = np.empty((N_CORES * bc, N_NETS, LOD), np.float32)
    for core, res in enumerate(results):
        ot = np.asarray(res["OT"], np.float32)          # [nt*18, bc]
        # rows: triple*18 + 6u + l -> net 3*triple + u
        o = ot.reshape(nt, 3, LOD, bc)                   # [T, u, l, b]
        o = o.transpose(3, 0, 1, 2).reshape(bc, nt * 3, LOD)
        out[core * bc:(core + 1) * bc] = o[:, :N_NETS, :]
    return out


_PROGRAM_CACHE = {}


def kernel(prior_lod, camera, W1, b1, W2, b2, W3, b3):
    from concourse.bass_utils import run_bass_kernel_spmd
    in_maps = build_host_tensors(prior_lod, camera, W1, b1, W2, b2, W3, b3)
    key = (BC, NSG)
    if key not in _PROGRAM_CACHE:
        _PROGRAM_CACHE[key] = build_program()
    nc = _PROGRAM_CACHE[key]
    res = run_bass_kernel_spmd(nc, in_maps, list(range(N_CORES)))
    return _unshard(res.results)


# revision 13
# speedup vs baseline: 1.2837x; 1.2837x over previous
"""Trainium2 Bass kernel for nn_Net_18966575579675 (dense_mlp).

722 independent tiny MLPs: per net n, per batch row b:
  x = [prior[b,n], camera[b,:]]            (11)
  h1 = relu(x @ W1[n] + b1[n])             (10)
  h2 = relu(h1 @ W2[n] + b2[n])            (10)
  out[b,n,:] = h2 @ W3[n] + b3[n]          (6)

Strategy: data-parallel over batch (8 cores x 1024 rows). On each core the
PE runs in 32x32 tiled mode (16 independent sub-arrays). Each tile handles
a "triple" (3 nets) as a block-diagonal matmul:
  L1: K=23 (10 cam + 12 prior-of-rowgroup + ones), M=30, zero-padding in
      the weight block selects the tile's own 3 prior rows.
  L2: K=31 (30 h1 + ones), M=30.
  L3: K=31 (30 h2 + ones), M=18 (bf16).
Biases ride on a constant ones-row. Relu is fused into the PSUM->SBUF
eviction (DVE tensor_scalar_max / ACT activation Relu).
Output leaves the device as [triple*18+v, batch]; the host transposes
during the unshard step.
"""

import sys

sys.path.insert(0, "/opt/trn_rl_repo")

import numpy as np
import ml_dtypes

import concourse.bass as bass
import concourse.bacc as bacc
import concourse.mybir as mybir
from concourse import tile
from concourse.tile import add_dep_helper

F32 = mybir.dt.float32
F32R = mybir.dt.float32r
BF16 = mybir.dt.bfloat16

# Problem constants
B = 8192
N_NETS = 722
CAM = 10
H = 10
LOD = 6
N_CORES = 8

# Kernel geometry (full-size)
BC = B // N_CORES      # batch per core (1024)
NSG = 16               # supergroups of 16 triples
NT = NSG * 16          # triples total (256 -> 768 nets, padded)
NQ = NSG * 4           # quads
NPAD = NT * 3          # padded net count (768)


def _pad_nets(a, npad):
    """Pad (or slice) axis 0 (net axis) with zeros up to npad."""
    if npad <= a.shape[0]:
        return a[:npad]
    pad = [(0, npad - a.shape[0])] + [(0, 0)] * (a.ndim - 1)
    return np.pad(a, pad)


def build_host_tensors(prior_lod, camera, W1, b1, W2, b2, W3, b3,
                       bc=BC, nsg=NSG, n_cores=N_CORES):
    """Build per-core DRAM input tensors (numpy). Returns list of dicts."""
    nt = nsg * 16
    nq = nsg * 4
    npad = nt * 3

    W1 = _pad_nets(np.asarray(W1, np.float32), npad)
    b1 = _pad_nets(np.asarray(b1, np.float32), npad)
    W2 = _pad_nets(np.asarray(W2, np.float32), npad)
    b2 = _pad_nets(np.asarray(b2, np.float32), npad)
    W3 = _pad_nets(np.asarray(W3, np.float32), npad)
    b3 = _pad_nets(np.asarray(b3, np.float32), npad)
    prior = np.asarray(prior_lod, np.float32)
    camera = np.asarray(camera, np.float32)

    # ---- weights (shared by all cores) ----
    # W1H[i, k, g, j, m]: rowgroup i, K=23 rows, supergroup g, tile j, M=30
    W1H = np.zeros((4, 23, nsg, 4, 32), np.float32)
    W1H[:, 22, :, :, 30] = 1.0  # ones-row passthrough keeps h1 row 30 == 1
    # triple index T = 16g + 4i + j ; nets 3T+u ; M col = 10u + h
    g_ = np.arange(nsg)[:, None, None]          # g
    i_ = np.arange(4)[None, :, None]            # i
    j_ = np.arange(4)[None, None, :]            # j
    T_ = 16 * g_ + 4 * i_ + j_                  # [g,i,j]
    for u in range(3):
        n_idx = 3 * T_ + u                      # [g,i,j]
        # cam rows 0..9: W1H[i, c, g, j, 10u+h] = W1[n, 1+c, h]
        blk = W1[n_idx, 1:, :]                  # [g,i,j,10,10]
        for c in range(CAM):
            W1H[:, c, :, :, 10 * u:10 * u + 10] = np.transpose(
                blk[:, :, :, c, :], (1, 0, 2, 3))
        # prior row 10 + 3j + u: weight W1[n, 0, h]
        pr = W1[n_idx, 0, :]                    # [g,i,j,10]
        for jj in range(4):
            W1H[:, 10 + 3 * jj + u, :, jj, 10 * u:10 * u + 10] = np.transpose(
                pr[:, :, jj, :], (1, 0, 2))
        # ones row 22: bias
        W1H[:, 22, :, :, 10 * u:10 * u + 10] = np.transpose(
            b1[n_idx], (1, 0, 2, 3))
    W1H = np.ascontiguousarray(W1H.reshape(4, 23, nsg * 4 * 32)).astype(
        ml_dtypes.bfloat16)

    # W2H[t, k, q, m]: rowgroup t (triple-in-quad), K=31, quad q, M=30
    W2H = np.zeros((4, 31, nq, 32), np.float32)
    W2H[:, 30, :, 30] = 1.0
    q_ = np.arange(nq)[None, :]
    t_ = np.arange(4)[:, None]
    T2 = 4 * q_ + t_                            # [t, q]
    for u in range(3):
        n_idx = 3 * T2 + u                      # [t, q]
        w = W2[n_idx]                           # [t, q, 10, 10]
        for h in range(H):
            W2H[:, 10 * u + h, :, 10 * u:10 * u + 10] = w[:, :, h, :]
        W2H[:, 30, :, 10 * u:10 * u + 10] = b2[n_idx]
    W2H = np.ascontiguousarray(W2H.reshape(4, 31, nq * 32)).astype(
        ml_dtypes.bfloat16)

    # W3H[w, k, T, m]: replicated across rowgroups w, K=31, M=18, bf16
    W3H = np.zeros((31, nt, 32), np.float32)
    T3 = np.arange(nt)
    for u in range(3):
        n_idx = 3 * T3 + u
        w = W3[n_idx]                           # [T, 10, 6]
        for h in range(H):
            W3H[10 * u + h, :, 6 * u:6 * u + 6] = w[:, h, :]
        W3H[30, :, 6 * u:6 * u + 6] = b3[n_idx]
    W3H = np.broadcast_to(W3H[None], (4, 31, nt, 32))
    W3H = np.ascontiguousarray(W3H.reshape(4, 31, nt * 32)).astype(
        ml_dtypes.bfloat16)

    # ---- per-core xin ----
    in_maps = []
    for core in range(n_cores):
        b0 = core * bc
        pr = prior[b0:b0 + bc]                  # [bc, 722]
        cam = camera[b0:b0 + bc]                # [bc, 10]
        xin = np.zeros((4, 23, nsg, bc), np.float32)
        xin[:, 0:10] = cam.T[None, :, None, :]
        xin[:, 22] = 1.0
        # prior rows 10 + 3j + u -> net 3*(16g+4i+j)+u
        for i in range(4):
            for jj in range(4):
                for u in range(3):
                    nets = 3 * (16 * np.arange(nsg) + 4 * i + jj) + u  # [g]
                    valid = nets < N_NETS
                    rows = np.zeros((nsg, bc), np.float32)
                    rows[valid] = pr[:, nets[valid]].T
                    xin[i, 10 + 3 * jj + u] = rows
        xin = np.ascontiguousarray(xin.reshape(4, 23, nsg * bc)).astype(
            ml_dtypes.bfloat16)
        in_maps.append({"xin": xin, "w1": W1H, "w2": W2H, "w3": W3H})
    return in_maps


def build_program(bc=BC, nsg=NSG, reps=0):
    """Build the per-core Bass program (SPMD; identical on all cores).

    reps>0 wraps the whole body in a For_i repeat loop (timing builds only).
    """
    nt = nsg * 16
    nq = nsg * 4
    hc = bc // 2   # chunk size (L1 free dim)
    qc = bc // 4   # window size (L2/L3 free dim)

    nc = bacc.Bacc(None)
    xin_d = nc.dram_tensor("xin", [4, 23, nsg * bc], BF16, kind="ExternalInput")
    w1_d = nc.dram_tensor("w1", [4, 23, nsg * 4 * 32], BF16, kind="ExternalInput")
    w2_d = nc.dram_tensor("w2", [4, 31, nq * 32], BF16, kind="ExternalInput")
    w3_d = nc.dram_tensor("w3", [4, 31, nt * 32], BF16, kind="ExternalInput")
    ot_d = nc.dram_tensor("OT", [nt * 18, bc], F32, kind="ExternalOutput")
    # view for the output DMA: (g, pr, p, t, v) rows, (w, c) cols
    ot_v = ot_d[:].rearrange(
        "(g pr p t v) (w c) -> g pr t v w p c",
        g=nsg, pr=2, p=2, t=4, v=18, w=4, c=qc)

    import contextlib
    with tile.TileContext(nc) as tc:
        with tc.tile_pool(name="fix", bufs=1) as fix, \
             tc.tile_pool(name="psum", bufs=2, space="PSUM") as pp, \
             (tc.For_i(0, reps, 1) if reps else contextlib.nullcontext()):
            X = fix.tile([128, nsg * bc], BF16, tag="X")
            W1s = fix.tile([128, nsg * 4 * 32], BF16, tag="W1s")
            W2s = fix.tile([128, nq * 32], BF16, tag="W2s")
            W3s = fix.tile([128, nt * 32], BF16, tag="W3s")
            h1 = [fix.tile([128, 4 * bc], BF16, tag=f"h1{x}", name=f"h1{x}") for x in "ab"]
            h2 = [fix.tile([128, 2 * bc], BF16, tag=f"h2{x}", name=f"h2{x}") for x in "ab"]
            osb = [fix.tile([128, 2 * bc], F32, tag=f"osb{x}", name=f"osb{x}") for x in "ab"]

            # ---- input DMAs ----
            for i in range(4):
                nc.sync.dma_start(out=X[32 * i:32 * i + 23, :], in_=xin_d[i])
                nc.sync.dma_start(out=W1s[32 * i:32 * i + 23, :], in_=w1_d[i])
                nc.sync.dma_start(out=W2s[32 * i:32 * i + 31, :], in_=w2_d[i])
                nc.sync.dma_start(out=W3s[32 * i:32 * i + 31, :], in_=w3_d[i])

            # ---- main loop over supergroups ----
            pe_prev = [None]
            for g in range(nsg):
                pg = g % 2
                h1sb, h2sb_, osb_ = h1[pg], h2, osb

                # L1: two chunks, 16 tiles each (diagonal waves: rgs rotate)
                Tts = []
                for c in range(2):
                    Tt = pp.tile([128, 2048], F32, tag="ps", name="psT1")
                    Tts.append(Tt)
                    for k in range(4):
                        for i in range(4):
                            j = (i + k) % 4
                            mm = nc.tensor.matmul(
                                Tt[32 * j:32 * j + 32,
                                   512 * i:512 * i + hc],
                                W1s[32 * i:32 * i + 23,
                                    (g * 4 + j) * 32:(g * 4 + j) * 32 + 32
                                    ],
                                X[32 * i:32 * i + 23,
                                  g * bc + c * hc:g * bc + (c + 1) * hc
                                  ],
                                tile_position=(32 * i, 32 * j),
                            )
                            if pe_prev[0] is not None:
                                add_dep_helper(mm.ins, pe_prev[0],
                                               reason="pe-rr")
                            pe_prev[0] = mm.ins
                # L1 evict (relu) -> h1sb ; split banks across DVE/ACT
                for c, Tt in enumerate(Tts):
                    src = Tt[:, :].rearrange("p (i x) -> p i x", i=4)[
                        :, :, 0:hc]
                    dst = h1sb[:, :].rearrange("p (i x) -> p i x", i=4)[
                        :, :, c * hc:(c + 1) * hc]
                    nc.vector.tensor_scalar_max(
                        dst[:, 0:2], src[:, 0:2], 0.0)
                    nc.scalar.activation(
                        dst[:, 2:4], src[:, 2:4],
                        mybir.ActivationFunctionType.Relu)

                # L2: 4 quads -> 2 psum tiles (pair: p = q%2)
                for pair in range(2):
                    T2t = pp.tile([128, 2048], F32, tag="ps", name="psT2")
                    for p in range(2):
                        i_blk = 2 * pair + p
                        q = 4 * g + i_blk
                        for k in range(4):
                            for t in range(4):
                                w = (t + k) % 4
                                mm = nc.tensor.matmul(
                                    T2t[32 * w:32 * w + 32,
                                        512 * t + qc * p:512 * t + qc * p + qc],
                                    W2s[32 * t:32 * t + 31,
                                        q * 32:q * 32 + 32],
                                    h1sb[32 * t:32 * t + 31,
                                         i_blk * bc + w * qc:
                                         i_blk * bc + (w + 1) * qc
                                         ],
                                    tile_position=(32 * t, 32 * w),
                                )
                                add_dep_helper(mm.ins, pe_prev[0],
                                               reason="pe-rr")
                                pe_prev[0] = mm.ins
                    # evict (relu, ->bf16) -> h2sb[pair]; bank-split
                    src = T2t[:, :].rearrange("p (t x) -> p t x", t=4)[
                        :, :, 0:hc]
                    dst = h2sb_[pair][:, :].rearrange(
                        "p (t x) -> p t x", t=4)[:, :, 0:hc]
                    nc.vector.tensor_scalar_max(
                        dst[:, 0:2], src[:, 0:2], 0.0)
                    nc.scalar.activation(
                        dst[:, 2:4], src[:, 2:4],
                        mybir.ActivationFunctionType.Relu)

                # L3: per pair, 8 triples (t, p) x 4 windows, bf16
                for pair in range(2):
                    T3t = pp.tile([128, 2048], F32, tag="ps", name="psT3")
                    for p in range(2):
                        for k in range(4):
                            for w in range(4):
                                t = (w + k) % 4
                                T_g = 4 * (4 * g + 2 * pair + p) + t
                                mm = nc.tensor.matmul(
                                    T3t[32 * t:32 * t + 32,
                                        512 * w + qc * p:512 * w + qc * p + qc],
                                    W3s[32 * w:32 * w + 31,
                                        T_g * 32:T_g * 32 + 32],
                                    h2sb_[pair][32 * w:32 * w + 31,
                                                hc * t + qc * p:
                                                hc * t + qc * p + qc],
                                    tile_position=(32 * w, 32 * t),
                                )
                                add_dep_helper(mm.ins, pe_prev[0],
                                               reason="pe-rr")
                                pe_prev[0] = mm.ins
                    # evict (copy) -> osb[pair]; bank-split
                    src = T3t[:, :].rearrange("p (w x) -> p w x", w=4)[
                        :, :, 0:hc]
                    dst = osb_[pair][:, :].rearrange(
                        "p (w x) -> p w x", w=4)[:, :, 0:hc]
                    nc.vector.tensor_scalar_add(
                        dst[:, 0:2], src[:, 0:2], 0.0)
                    nc.scalar.activation(
                        dst[:, 2:4], src[:, 2:4],
                        mybir.ActivationFunctionType.Copy)

                    # output DMA: per t, [18, w, p, c]
                    for t in range(4):
                        src_d = osb_[pair][32 * t:32 * t + 18, :].rearrange(
                            "v (w p c) -> v w p c", w=4, p=2)
                        nc.sync.dma_start(
                            out=ot_v[g, pair, t], in_=src_d)
    nc.finalize()
    return nc


def _unshard(results, bc=BC, nsg=NSG):
    nt = nsg * 16
    out = np.empty((N_CORES * bc, N_NETS, LOD), np.float32)
    for core, res in enumerate(results):
        ot = np.asarray(res["OT"], np.float32)          # [nt*18, bc]
        # rows: triple*18 + 6u + l -> net 3*triple + u
        o = ot.reshape(nt, 3, LOD, bc)                   # [T, u, l, b]
        o = o.transpose(3, 0, 1, 2).reshape(bc, nt * 3, LOD)
        out[core * bc:(core + 1) * bc] = o[:, :N_NETS, :]
    return out


_PROGRAM_CACHE = {}


def kernel(prior_lod, camera, W1, b1, W2, b2, W3, b3):
    from concourse.bass_utils import run_bass_kernel_spmd
    in_maps = build_host_tensors(prior_lod, camera, W1, b1, W2, b2, W3, b3)
    key = (BC, NSG)
    if key not in _PROGRAM_CACHE:
        _PROGRAM_CACHE[key] = build_program()
    nc = _PROGRAM_CACHE[key]
    res = run_bass_kernel_spmd(nc, in_maps, list(range(N_CORES)))
    return _unshard(res.results)
